# revision 1
# baseline (speedup 1.0000x reference)
"""Trainium2 Bass kernel for a single-head cross-attention block.

Reference computation (per batch b of B=128):
    q = input[b] @ Wq            # [T,H]   T=512, C=384, H=64
    k = x[b] @ Wk                # [T,H]
    v = x[b] @ Wv                # [T,H]
    S = (q @ k.T) * C**-0.5      # [T,T], causal mask
    P = softmax(S, axis=-1)
    out[b] = P @ v               # [T,H]

Strategy: data-parallel over 8 NeuronCores (16 batches each). Host-side we
pre-transpose input/x to [C,T] (the PE contracts along partitions, so the
projections need C on partitions) and cast to bf16. On device, per batch:

  - qT/kT = Wq'.T @ inpT / Wk'.T @ xT        -> PSUM [64,512] each
  - v[t]  = xT[:,tchunk].T @ Wv'             -> PSUM [128,64] x4
  - S^T[m] = kT[:,mchunk].T @ qT             -> PSUM [128, 512-128m]
    (S^T layout [k,q]: causal keeps q >= k, so chunk m only needs
     columns 128m..512; the diagonal 128x128 block is masked with a
     precomputed upper-triangular 0/1 tile)
  - E = exp(S^T * scale) on ScalarE (scale fused into the activation)
    No max-subtraction needed: scores are N(0, 0.41^2)-ish, |s|<~3.
  - out_ps[t] += E[m][:,tchunk].T @ [v[m] | 1]   (ones column makes the
    softmax denominator fall out of the same matmuls)
  - out = out_ps[:, :H] * (1/denom)  per-partition scalar, then DMA out.
"""

import numpy as np
import ml_dtypes

import concourse.bass as bass
import concourse.tile as tile
import concourse.mybir as mybir
from concourse.vector_clock import ScopedClock
from concourse.bass_utils import run_bass_kernel_spmd
from concourse.masks import make_upper_triangular

N_CORES = 8
B, T, C, H = 128, 512, 384, 64
BPC = B // N_CORES          # batches per core
CK = C // 128               # contraction chunks for projections
TK = T // 128               # T chunks
SCALE = float(C) ** -0.5
BF16 = mybir.dt.bfloat16
F32 = mybir.dt.float32
EXP = mybir.ActivationFunctionType.Exp

_bf16 = ml_dtypes.bfloat16


def _split_multi_waits(nc: bass.Bass):
    """walrus in this build encodes at most ONE sync-wait per instruction.
    Tile's wait-assignment can attach several. Move the extras onto
    same-engine NOPs inserted immediately before each instruction —
    identical semantics (the engine blocks on the NOP waits first)."""
    n = 0
    for bb in nc.m.functions[0].blocks:
        new_insts = []
        for inst in bb.instructions:
            si = inst.sync_info
            waits = list(si.on_wait) if si and si.on_wait else []
            if len(waits) > 1:
                for w in waits[:-1]:
                    nop = mybir.InstNoOp(name=f"WSPLIT-{n}", ins=[], outs=[])
                    n += 1
                    nop.engine = inst.engine
                    nop.sync_info = mybir.SyncInfo(on_wait=[w], on_update=[])
                    new_insts.append(nop)
                si.on_wait = waits[-1:]
            new_insts.append(inst)
        bb.instructions[:] = new_insts


def build_kernel() -> bass.Bass:
    nc = bass.Bass()
    inpT = nc.dram_tensor("inpT", [BPC, C, T], BF16, kind="ExternalInput")
    xT = nc.dram_tensor("xT", [BPC, C, T], BF16, kind="ExternalInput")
    wq = nc.dram_tensor("wq", [C, H], BF16, kind="ExternalInput")
    wk = nc.dram_tensor("wk", [C, H], BF16, kind="ExternalInput")
    wv = nc.dram_tensor("wv", [C, H], BF16, kind="ExternalInput")
    out = nc.dram_tensor("out", [BPC, T, H], F32, kind="ExternalOutput")

    with tile.TileContext(nc) as tc:
        with (
            tc.tile_pool(name="const", bufs=1) as const_pool,
            tc.tile_pool(name="inputs", bufs=3) as in_pool,
            tc.tile_pool(name="work", bufs=2) as sb_pool,
            tc.tile_pool(name="qk_ps", bufs=1, space="PSUM") as qk_psum,
            tc.tile_pool(name="v_ps", bufs=1, space="PSUM") as v_psum,
            tc.tile_pool(name="st_ps", bufs=3, space="PSUM") as st_psum,
            tc.tile_pool(name="o_ps", bufs=2, space="PSUM") as o_psum,
        ):
            # Constants: weights as [128, CK, H] (C-chunk on partitions), and
            # the upper-triangular (incl. diagonal) 0/1 mask for the causal
            # diagonal blocks of S^T.
            wq_sb = const_pool.tile([128, CK, H], BF16, tag="wq")
            nc.sync.dma_start(wq_sb[:], wq[:, :].rearrange("(c p) h -> p c h", p=128))
            wk_sb = const_pool.tile([128, CK, H], BF16, tag="wk")
            nc.sync.dma_start(wk_sb[:], wk[:, :].rearrange("(c p) h -> p c h", p=128))
            wv_sb = const_pool.tile([128, CK, H], BF16, tag="wv")
            nc.sync.dma_start(wv_sb[:], wv[:, :].rearrange("(c p) h -> p c h", p=128))
            tri = const_pool.tile([128, 128], BF16, tag="tri")
            make_upper_triangular(nc, tri[:], val=1.0, diag=True)

            for b in range(BPC):
                it = in_pool.tile([128, CK, T], BF16, tag="inpT")
                nc.sync.dma_start(it[:], inpT[b].rearrange("(c p) t -> p c t", p=128))
                xt = in_pool.tile([128, CK, T], BF16, tag="xT")
                nc.sync.dma_start(xt[:], xT[b].rearrange("(c p) t -> p c t", p=128))

                # qT | kT in one 2-bank PSUM tile [64, 2T]. Casts are split
                # so the q-cast overlaps the k-projections and the k-cast
                # overlaps the v-matmuls (keeps PE from stalling on S^T).
                qk_ps = qk_psum.tile([H, 2 * T], F32, tag="qk")
                qk_sb = sb_pool.tile([H, 2 * T], BF16, tag="qk_sb")
                for c in range(CK):
                    nc.tensor.matmul(
                        qk_ps[:, 0:T], wq_sb[:, c, :], it[:, c, :],
                        start=(c == 0), stop=(c == CK - 1),
                    )
                nc.vector.tensor_copy(qk_sb[:, 0:T], qk_ps[:, 0:T])
                for c in range(CK):
                    nc.tensor.matmul(
                        qk_ps[:, T:2 * T], wk_sb[:, c, :], xt[:, c, :],
                        start=(c == 0), stop=(c == CK - 1),
                    )
                nc.vector.tensor_copy(qk_sb[:, T:2 * T], qk_ps[:, T:2 * T])

                # v chunks [128, H] x TK in one PSUM bank
                v_ps = v_psum.tile([128, TK, H], F32, tag="v")
                for t in range(TK):
                    for c in range(CK):
                        nc.tensor.matmul(
                            v_ps[:, t, :],
                            xt[:, c, 128 * t:128 * (t + 1)],
                            wv_sb[:, c, :],
                            start=(c == 0), stop=(c == CK - 1),
                        )
                v_sb = sb_pool.tile([128, TK, H + 1], BF16, tag="v_sb")
                nc.vector.tensor_copy(v_sb[:, :, 0:H], v_ps[:])
                nc.gpsimd.memset(v_sb[:, :, H], 1.0)

                qT = qk_sb[:, 0:T]
                kT = qk_sb[:, T:2 * T]

                # S^T chunks -> exp -> (mask diagonal block)
                e_tiles = []
                for m in range(TK):
                    n0 = 128 * m
                    st_ps = st_psum.tile([128, T], F32, tag="st")
                    nc.tensor.matmul(
                        st_ps[:, n0:T],
                        kT[:, n0:n0 + 128],
                        qT[:, n0:T],
                        start=True, stop=True,
                    )
                    e = sb_pool.tile([128, T], BF16, tag=f"e{m}")
                    nc.scalar.activation(e[:, n0:T], st_ps[:, n0:T], EXP, scale=SCALE)
                    nc.vector.tensor_mul(e[:, n0:n0 + 128], e[:, n0:n0 + 128], tri[:])
                    e_tiles.append(e)

                # out accumulation over k-chunks; col H carries the denominator
                o_ps = o_psum.tile([128, TK, H + 1], F32, tag="o")
                for t in range(TK):
                    for m in range(t + 1):
                        nc.tensor.matmul(
                            o_ps[:, t, :],
                            e_tiles[m][:, 128 * t:128 * (t + 1)],
                            v_sb[:, m, :],
                            start=(m == 0), stop=(m == t),
                        )

                # normalize and store
                recip = sb_pool.tile([128, TK], F32, tag="recip")
                nc.vector.reciprocal(recip[:], o_ps[:, :, H])
                o_sb = sb_pool.tile([128, TK, H], F32, tag="o_sb")
                for t in range(TK):
                    nc.scalar.mul(o_sb[:, t, :], o_ps[:, t, 0:H], recip[:, t:t + 1])
                nc.sync.dma_start(
                    out[b].rearrange("(t p) h -> p t h", p=128), o_sb[:]
                )
    _split_multi_waits(nc)
    return nc


_cached_nc = None


def kernel(input: np.ndarray, x: np.ndarray, Wq: np.ndarray, Wk: np.ndarray,
           Wv: np.ndarray) -> np.ndarray:
    global _cached_nc

    input = np.asarray(input, dtype=np.float32)
    x = np.asarray(x, dtype=np.float32)
    inpT = np.transpose(input, (0, 2, 1)).astype(_bf16)   # [B, C, T] bf16
    xT = np.transpose(x, (0, 2, 1)).astype(_bf16)
    wq_b = np.asarray(Wq, dtype=np.float32).astype(_bf16)
    wk_b = np.asarray(Wk, dtype=np.float32).astype(_bf16)
    wv_b = np.asarray(Wv, dtype=np.float32).astype(_bf16)

    if _cached_nc is None:
        _cached_nc = build_kernel()
    nc = _cached_nc

    in_maps = []
    for c in range(N_CORES):
        sl = slice(c * BPC, (c + 1) * BPC)
        in_maps.append({
            "inpT": np.ascontiguousarray(inpT[sl]),
            "xT": np.ascontiguousarray(xT[sl]),
            "wq": wq_b, "wk": wk_b, "wv": wv_b,
        })

    res = run_bass_kernel_spmd(nc, in_maps, core_ids=list(range(N_CORES)))
    out = np.concatenate([r["out"] for r in res.results], axis=0)
    return out.astype(np.float32)



# revision 4
# speedup vs baseline: 1.5867x; 1.5867x over previous
"""Trainium2 Bass kernel for a single-head cross-attention block.

Reference computation (per batch b of B=128):
    q = input[b] @ Wq            # [T,H]   T=512, C=384, H=64
    k = x[b] @ Wk                # [T,H]
    v = x[b] @ Wv                # [T,H]
    S = (q @ k.T) * C**-0.5      # [T,T], causal mask
    P = softmax(S, axis=-1)
    out[b] = P @ v               # [T,H]

Data-parallel over 8 NeuronCores (16 batches each).  Key structure:

  * Host pre-packs input+x per batch into ONE DRAM blob laid out so every
    SBUF partition's slice is a single 6 KiB contiguous run -> the DMA ring
    moves 128 big descriptors per batch instead of ~770 one-KiB ones
    (packet dispatch was the baseline bottleneck).
  * k and v projections are fused into one matmul pass using a stacked
    [Wk|Wv] stationary tile: out [128,T] holds kT on partitions 0-63 and
    vT on 64-127 (halves the kv projection PE cost).  Odd batches use
    [Wv|Wk] so kT lands on partitions 64-127 - this matches the q
    projection of odd batches which is PSUM-stacked on partitions 64-127
    (two batches share one PSUM bank + one PSUM->SBUF copy).
  * v is re-oriented [t,h] with 4 tiny PE transposes (bf16, via identity).
  * S^T chunks exploit causality (chunk m covers queries >= 128m);
    exp runs on ScalarE in 2 instructions (chunk 0, chunks 1+3+2 packed
    into one PSUM tile); diagonal-block masks multiply by a 0/1 upper-
    triangular tile on DVE (4x mode: bf16, SBUF-only).
  * P@V accumulates with an extra ones-column on v so the softmax
    denominator falls out of the same matmuls; normalization is a
    broadcast multiply by the reciprocal.
  * Software pipelining: q projections run two batches ahead, kv one
    batch ahead, so the PE stream stays dense while exp/mask chase it.
"""

import numpy as np
import ml_dtypes

import concourse.bass as bass
import concourse.tile as tile
import concourse.mybir as mybir
from concourse.bass import broadcast_tensor_aps
from concourse.bass_utils import run_bass_kernel_spmd
from concourse.masks import make_upper_triangular, make_identity

N_CORES = 8
B, T, C, H = 128, 512, 384, 64
BPC = B // N_CORES          # batches per core
CK = C // 128               # contraction chunks for projections
TK = T // 128               # T chunks
SCALE = float(C) ** -0.5
BF16 = mybir.dt.bfloat16
F32 = mybir.dt.float32
EXP = mybir.ActivationFunctionType.Exp
MULT = mybir.AluOpType.mult

_bf16 = ml_dtypes.bfloat16

# weight tile column layout: [Wk|Wv] (even), [Wv|Wk] (odd), Wq
WKV_E, WKV_O, WQ0 = 0, 128, 256
WCOLS = 320


def _split_multi_waits(nc: bass.Bass):
    """walrus in this build encodes at most ONE sync-wait per instruction.
    Tile's wait-assignment can attach several. Move the extras onto
    same-engine NOPs inserted immediately before each instruction —
    identical semantics (the engine blocks on the NOP waits first)."""
    n = 0
    for bb in nc.m.functions[0].blocks:
        new_insts = []
        for inst in bb.instructions:
            si = inst.sync_info
            waits = list(si.on_wait) if si and si.on_wait else []
            if len(waits) > 1:
                for w in waits[:-1]:
                    nop = mybir.InstNoOp(name=f"WSPLIT-{n}", ins=[], outs=[])
                    n += 1
                    nop.engine = inst.engine
                    nop.sync_info = mybir.SyncInfo(on_wait=[w], on_update=[])
                    new_insts.append(nop)
                si.on_wait = waits[-1:]
            new_insts.append(inst)
        bb.instructions[:] = new_insts


def build_kernel(split_waits: bool = True) -> bass.Bass:
    nc = bass.Bass()
    # fused[b, p, 0:1536] = input[b, :, (c,p)] chunks; [b, p, 1536:3072] = x
    fused = nc.dram_tensor("fused", [BPC, 128, 2 * CK * T], BF16,
                           kind="ExternalInput")
    w = nc.dram_tensor("w", [128, CK * WCOLS], BF16, kind="ExternalInput")
    out = nc.dram_tensor("out", [BPC, 128, TK * H], F32, kind="ExternalOutput")

    with tile.TileContext(nc) as tc:
        with (
            tc.tile_pool(name="const", bufs=1) as const_pool,
            tc.tile_pool(name="ld", bufs=4) as ld_pool,
            tc.tile_pool(name="kv", bufs=2) as kv_pool,
            tc.tile_pool(name="qsb", bufs=2) as q_pool,
            tc.tile_pool(name="vsb", bufs=2) as v_pool,
            tc.tile_pool(name="e", bufs=2) as e_pool,
            tc.tile_pool(name="osb", bufs=2) as o_pool,
            tc.tile_pool(name="rcp", bufs=2) as r_pool,
            tc.tile_pool(name="q_ps", bufs=1, space="PSUM") as q_psum,
            tc.tile_pool(name="kv_ps", bufs=1, space="PSUM") as kv_psum,
            tc.tile_pool(name="vt_ps", bufs=1, space="PSUM") as vt_psum,
            tc.tile_pool(name="st_ps", bufs=1, space="PSUM") as st_psum,
            tc.tile_pool(name="o_ps", bufs=2, space="PSUM") as o_psum,
        ):
            # ---- constants ----
            w_sb = const_pool.tile([128, CK, WCOLS], BF16, tag="w")
            nc.scalar.dma_start(w_sb[:], w[:].rearrange("p (c f) -> p c f", c=CK))
            tri = const_pool.tile([128, 128], BF16, tag="tri")
            make_upper_triangular(nc, tri[:], val=1.0, diag=True)
            tripair = const_pool.tile([128, 2, 128], BF16, tag="tripair")
            make_upper_triangular(nc, tripair[:, 0, :], val=1.0, diag=True)
            make_upper_triangular(nc, tripair[:, 1, :], val=1.0, diag=True)
            # identity stacked twice so both parities can transpose from
            # their base partition (0 or 64)
            ident = const_pool.tile([128, 64], BF16, tag="ident")
            make_identity(nc, ident[0:64, :])
            make_identity(nc, ident[64:128, :])

            lds, kvs, qps_t, qsbs = {}, {}, {}, {}

            def emit_ld(i):
                t_ = ld_pool.tile([128, 2, CK, T], BF16, tag="ld", name="ld")
                nc.sync.dma_start(
                    t_[:], fused[i].rearrange("p (s c t) -> p s c t", s=2, c=CK))
                lds[i] = t_

            def emit_kv(i):
                # stationary [Wk|Wv] (even i) or [Wv|Wk] (odd i)
                ps = kv_psum.tile([128, T], F32, tag="kv")
                wc = WKV_E if i % 2 == 0 else WKV_O
                for c in range(CK):
                    nc.tensor.matmul(
                        ps[:, :], w_sb[:, c, wc:wc + 128], lds[i][:, 1, c, :],
                        start=(c == 0), stop=(c == CK - 1))
                return ps

            def emit_kv_copy(i, ps):
                sb = kv_pool.tile([128, T], BF16, tag="kv_sb")
                nc.vector.tensor_copy(sb[:, 0:256], ps[:, 0:256])
                nc.scalar.copy(sb[:, 256:512], ps[:, 256:512])
                kvs[i] = sb

            def emit_q(i):
                # batches 2k/2k+1 stack into one PSUM tile at partition 0/64
                pair = i // 2
                if i % 2 == 0:
                    qps_t[pair] = q_psum.tile([128, T], F32, tag="q", name="q_ps")
                ps = qps_t[pair]
                pbase = 64 * (i % 2)
                for c in range(CK):
                    nc.tensor.matmul(
                        ps[pbase:pbase + 64, :], w_sb[:, c, WQ0:WQ0 + 64],
                        lds[i][:, 0, c, :],
                        start=(c == 0), stop=(c == CK - 1))

            def emit_qcopy(pair):
                sb = q_pool.tile([128, T], BF16, tag="q_sb")
                nc.vector.tensor_copy(sb[:], qps_t[pair][:])
                qsbs[pair] = sb
                del qps_t[pair]

            def emit_vt(i):
                # transpose vT [64, T] -> v [t, h] chunks (PE, bf16)
                vb = 64 if i % 2 == 0 else 0
                ps = vt_psum.tile([128, TK, H], BF16, tag="vt")
                for tk in range(TK):
                    nc.tensor.transpose(
                        ps[:, tk, :], kvs[i][vb:vb + 64, 128 * tk:128 * (tk + 1)],
                        ident[vb:vb + 64, :])
                return ps

            def emit_vcopy(i, vt_ps):
                sb = v_pool.tile([128, TK, H + 1], BF16, tag="v_sb")
                nc.gpsimd.memset(sb[:, :, H], 1.0)
                nc.vector.tensor_copy(sb[:, :, 0:H], vt_ps[:])
                return sb

            def emit_S(i):
                kb = 0 if i % 2 == 0 else 64
                st0 = st_psum.tile([128, T], F32, tag="st0")
                # chunks 1,3,2 packed: m1 cols 0:384 | m3 384:512 | m2 512:768
                st123 = st_psum.tile([128, 768], F32, tag="st123")
                kv_sb, q_sb = kvs[i], qsbs[i // 2]
                nc.tensor.matmul(st0[:, :], kv_sb[kb:kb + 64, 0:128],
                                 q_sb[kb:kb + 64, :], start=True, stop=True)
                nc.tensor.matmul(st123[:, 0:384], kv_sb[kb:kb + 64, 128:256],
                                 q_sb[kb:kb + 64, 128:T], start=True, stop=True)
                nc.tensor.matmul(st123[:, 384:512], kv_sb[kb:kb + 64, 384:512],
                                 q_sb[kb:kb + 64, 384:T], start=True, stop=True)
                nc.tensor.matmul(st123[:, 512:768], kv_sb[kb:kb + 64, 256:384],
                                 q_sb[kb:kb + 64, 256:T], start=True, stop=True)
                return st0, st123

            def emit_PV(i, e0, e123, v_sb):
                ps = o_psum.tile([128, TK, H + 1], F32, tag="o")
                for t in range(TK):
                    for m in range(t + 1):
                        if m == 0:
                            lhsT = e0[:, 128 * t:128 * (t + 1)]
                        elif m == 1:
                            lhsT = e123[:, 128 * (t - 1):128 * t]
                        elif m == 2:
                            lhsT = e123[:, 512 + 128 * (t - 2):512 + 128 * (t - 1)]
                        else:
                            lhsT = e123[:, 384:512]
                        nc.tensor.matmul(ps[:, t, :], lhsT, v_sb[:, m, :],
                                         start=(m == 0), stop=(m == t))
                return ps

            # ---- prologue ----
            for i in range(3):
                emit_ld(i)
            kv_ps0 = emit_kv(0)
            emit_kv_copy(0, kv_ps0)
            emit_q(0)
            emit_q(1)
            emit_qcopy(0)

            # ---- steady-state loop ----
            for b in range(BPC):
                if b + 3 < BPC:
                    emit_ld(b + 3)
                vt_ps = emit_vt(b)
                v_sb = emit_vcopy(b, vt_ps)
                st0, st123 = emit_S(b)

                e0 = e_pool.tile([128, T], BF16, tag="e0")
                e123 = e_pool.tile([128, 768], BF16, tag="e123")
                nc.scalar.activation(e0[:], st0[:], EXP, scale=SCALE)
                nc.vector.tensor_mul(e0[:, 0:128], e0[:, 0:128], tri[:])
                nc.scalar.activation(e123[:], st123[:], EXP, scale=SCALE)

                if b + 1 < BPC:
                    ps = emit_kv(b + 1)
                    emit_kv_copy(b + 1, ps)
                    del lds[b]  # consumed

                # diagonal-block masks: m1 at cols 0:128, m3|m2 at 384:640
                nc.vector.tensor_mul(e123[:, 0:128], e123[:, 0:128], tri[:])
                dia, trp = broadcast_tensor_aps(
                    e123[:, 384:640].rearrange("p (u v) -> p u v", u=2),
                    tripair[:, :, :])
                nc.vector.tensor_mul(dia, dia, trp)

                if b + 2 < BPC:
                    emit_q(b + 2)
                    if (b + 2) % 2 == 1:
                        emit_qcopy((b + 2) // 2)

                o_ps = emit_PV(b, e0, e123, v_sb)

                recip = r_pool.tile([128, TK, 1], F32, tag="recip")
                nc.vector.reciprocal(recip[:, :, 0], o_ps[:, :, H])
                o_sb = o_pool.tile([128, TK, H], F32, tag="o_sb")
                src, rcp = broadcast_tensor_aps(o_ps[:, :, 0:H], recip[:])
                nc.vector.scalar_tensor_tensor(
                    o_sb[:], src, 1.0, rcp, op0=MULT, op1=MULT)
                nc.scalar.dma_start(
                    out[b].rearrange("p (t h) -> p t h", t=TK), o_sb[:])

    if split_waits:
        _split_multi_waits(nc)
    return nc


_cached_nc = None


def _pack_inputs(input, x, Wq, Wk, Wv):
    """Host-side repack: per-batch per-partition contiguous 6KiB blobs."""
    input = np.asarray(input, dtype=np.float32)
    x = np.asarray(x, dtype=np.float32)
    # [b, t, c*128+p] -> [b, p, s, c, t]
    fused = np.empty((B, 128, 2, CK, T), dtype=_bf16)
    fused[:, :, 0] = input.transpose(0, 2, 1).reshape(B, CK, 128, T).transpose(0, 2, 1, 3)
    fused[:, :, 1] = x.transpose(0, 2, 1).reshape(B, CK, 128, T).transpose(0, 2, 1, 3)
    fused = fused.reshape(B, 128, 2 * CK * T)

    Wq = np.asarray(Wq, dtype=np.float32)
    Wk = np.asarray(Wk, dtype=np.float32)
    Wv = np.asarray(Wv, dtype=np.float32)
    w_all = np.concatenate(
        [np.concatenate([Wk, Wv], 1), np.concatenate([Wv, Wk], 1), Wq], axis=1)
    w_host = np.ascontiguousarray(
        w_all.reshape(CK, 128, WCOLS).transpose(1, 0, 2).reshape(128, CK * WCOLS)
    ).astype(_bf16)
    return fused, w_host


def make_in_maps(input, x, Wq, Wk, Wv):
    fused, w_host = _pack_inputs(input, x, Wq, Wk, Wv)
    in_maps = []
    for c in range(N_CORES):
        sl = slice(c * BPC, (c + 1) * BPC)
        in_maps.append({
            "fused": np.ascontiguousarray(fused[sl]),
            "w": w_host,
        })
    return in_maps


def _unpack_out(res_outs):
    # out [BPC, 128, TK*H]: [b, p, t*64+h] = OUT[b, 128t+p, h]
    full = np.concatenate(res_outs, axis=0)               # [B, 128, TK*H]
    full = full.reshape(B, 128, TK, H).transpose(0, 2, 1, 3).reshape(B, T, H)
    return np.ascontiguousarray(full.astype(np.float32))


def kernel(input: np.ndarray, x: np.ndarray, Wq: np.ndarray, Wk: np.ndarray,
           Wv: np.ndarray) -> np.ndarray:
    global _cached_nc
    if _cached_nc is None:
        _cached_nc = build_kernel()
    nc = _cached_nc

    in_maps = make_in_maps(input, x, Wq, Wk, Wv)
    res = run_bass_kernel_spmd(nc, in_maps, core_ids=list(range(N_CORES)))
    return _unpack_out([r["out"] for r in res.results])
